# revision 2
# baseline (speedup 1.0000x reference)
"""Trainium2 Bass kernel for nn_FragAttention (segment_reduce).

Reference computation (S=128, B=512, D=512, G=S-1=127):
    xb     = transpose(x, (1,0,2))            # (B, S, D)
    xm     = xb * (~src_mask)[:, :, None]     # zero padded tokens
    left   [b,g,d] = sum_{s<=g} xm[b,s,d]     # masked prefix sums
    right  [b,g,d] = sum_{s>g}  xm[b,s,d]
    out    = concat([left, right], axis=2)    # (B, G, 2D)

Strategy: pure data parallel over B across 8 cores (64 batches each).
The pad mask is folded into x on the host (exact: multiply by 0/1), and
x is cast to bf16 on the host — halves input HBM traffic; the 0/1
triangular weights are exact in bf16 so only x's mantissa truncation
matters (~2e-3 rel err, gate is 2e-2). Per batch the prefix/suffix sums
are computed on the TensorEngine as two matmuls against constant 0/1
triangular matrices (contraction over S=128 on partitions, f32 PSUM
accumulate), then PSUM->SBUF copies (DVE for even batches, ACT for odd)
cast the result to bf16. The host upcasts the returned bf16 block.

DMA (the roofline resource: ~25 MB/core at ~358 GB/s HBM): both
directions use HWDGE, which sprays each transfer's per-partition
descriptors across all 16 SDMA engines (~300 GB/s measured). SWDGE
(gpsimd) was the previous bottleneck - it binds each DMA to a single
engine (~22 GB/s) and emits a 4-byte completion write per 512B chunk.
Output is written g-major (out[g, b, 2D]) so one partition row is a
128 KB contiguous DRAM run: OUT_CHUNK=16 batches -> 127 descriptors of
32 KB per DMA. Writes issue on the sync (SP) HWDGE ring, input loads on
the scalar (ACT) ring so descriptor generation runs in parallel.
"""

import numpy as np
import ml_dtypes

import concourse.bass as bass
import concourse.mybir as mybir
from concourse import bacc
from concourse.tile import TileContext
from concourse.bass_utils import run_bass_kernel_spmd

S, B, D = 128, 512, 512
G = S - 1
N_CORES = 8
BL = B // N_CORES  # 64 batches per core

IN_CHUNK = 16   # batches per input DMA  (16 KB per-partition descriptors)
OUT_CHUNK = 16  # batches per output DMA (32 KB per-partition descriptors)

_NC_CACHE = None


def _build_bass() -> bass.Bass:
    nc = bacc.Bacc()
    f32 = mybir.dt.float32
    bf16 = mybir.dt.bfloat16

    x_in = nc.declare_dram_parameter("x", [S, BL, D], bf16, isOutput=False)
    # tri[:, 0:128] = upper (incl diag)  tri[s,g] = 1 if s <= g  -> prefix sums
    # tri[:, 128:256] = strictly lower   tri[s,g] = 1 if s >  g  -> suffix sums
    t_in = nc.declare_dram_parameter("tri", [S, 2 * S], bf16, isOutput=False)
    # g-major per-core output: partition row g maps to a contiguous DRAM run,
    # host transposes (G, BL, 2D) -> (BL, G, 2D) while gathering.
    out = nc.declare_dram_parameter("out", [G, BL, 2 * D], bf16, isOutput=True)

    with TileContext(nc) as tc:
        with (
            tc.tile_pool(name="const", bufs=1) as cpool,
            tc.tile_pool(name="xin", bufs=2) as xpool,
            tc.tile_pool(name="outs", bufs=2) as opool,
            tc.tile_pool(name="psum", bufs=2, space="PSUM") as ppool,
        ):
            tri = cpool.tile([S, 2 * S], bf16)
            nc.sync.dma_start(out=tri[:], in_=t_in[:])
            ut = tri[:, 0:S]        # (128, 128) stationary, left sums
            lt = tri[:, S : 2 * S]  # (128, 128) stationary, right sums

            def per_pair(xt, ot, j, k):
                """2 batches (j, j+1) of xt -> slots (k, k+1) of ot.

                One 4-bank PSUM tile takes all 4 matmuls; DVE copies batch j,
                ACT copies batch j+1 - one copy op per batch total.
                """
                ps = ppool.tile([S, 4, D], f32)  # 4 adjacent banks
                for h, (b, tri_) in enumerate(
                    [(j, ut), (j, lt), (j + 1, ut), (j + 1, lt)]
                ):
                    nc.tensor.matmul(out=ps[:, h, :], lhsT=tri_, rhs=xt[:, b, :],
                                     start=True, stop=True)
                nc.vector.tensor_copy(
                    out=ot[0:G, k, :].rearrange("g (h d) -> g h d", h=2),
                    in_=ps[0:G, 0:2, :],
                )
                nc.scalar.activation(
                    out=ot[0:G, k + 1, :].rearrange("g (h d) -> g h d", h=2),
                    in_=ps[0:G, 2:4, :],
                    func=mybir.ActivationFunctionType.Copy,
                )

            for b0 in range(0, BL, OUT_CHUNK):
                ot = opool.tile([S, OUT_CHUNK, 2 * D], bf16)
                for c0 in range(b0, b0 + OUT_CHUNK, IN_CHUNK):
                    xt = xpool.tile([S, IN_CHUNK, D], bf16)
                    # input loads on the ACT HWDGE ring (writes own the SP ring)
                    nc.scalar.dma_start(
                        out=xt[:], in_=x_in[:, c0 : c0 + IN_CHUNK, :])
                    for j in range(0, IN_CHUNK, 2):
                        per_pair(xt, ot, j, c0 - b0 + j)
                nc.sync.dma_start(
                    out=out[:, b0 : b0 + OUT_CHUNK, :], in_=ot[0:G, :, :],
                )
    nc.finalize()  # runs the Bacc pass pipeline (reg alloc, wait splitting)
    return nc


def _get_nc() -> bass.Bass:
    global _NC_CACHE
    if _NC_CACHE is None:
        _NC_CACHE = _build_bass()
    return _NC_CACHE


def _make_in_maps(x: np.ndarray, src_mask: np.ndarray) -> list[dict]:
    x = np.asarray(x, dtype=np.float32)
    src_mask = np.asarray(src_mask)
    assert x.shape == (S, B, D), x.shape
    assert src_mask.shape == (B, S), src_mask.shape

    valid = (~src_mask.astype(bool)).astype(np.float32).T  # (S, B)
    xm = (x * valid[:, :, None]).astype(ml_dtypes.bfloat16)
    tri = np.concatenate(
        [
            np.triu(np.ones((S, S), np.float32)),       # s <= g
            np.tril(np.ones((S, S), np.float32), -1),   # s >  g
        ],
        axis=1,
    ).astype(ml_dtypes.bfloat16)

    in_maps = []
    for i in range(N_CORES):
        sl = slice(i * BL, (i + 1) * BL)
        in_maps.append(
            {
                "x": np.ascontiguousarray(xm[:, sl, :]),
                "tri": tri,
            }
        )
    return in_maps


def _assemble(results: list[dict]) -> np.ndarray:
    full = np.empty((B, G, 2 * D), dtype=np.float32)
    for i in range(N_CORES):
        full[i * BL : (i + 1) * BL] = (
            results[i]["out"].transpose(1, 0, 2).astype(np.float32)
        )
    return full


def kernel(x: np.ndarray, src_mask: np.ndarray) -> np.ndarray:
    in_maps = _make_in_maps(x, src_mask)
    res = run_bass_kernel_spmd(_get_nc(), in_maps, core_ids=list(range(N_CORES)))
    return _assemble(res.results)


# revision 6
# speedup vs baseline: 3.5700x; 3.5700x over previous
"""Trainium2 Bass kernel for nn_FragAttention (segment_reduce).

Reference computation (S=128, B=512, D=512, G=S-1=127):
    xb     = transpose(x, (1,0,2))            # (B, S, D)
    xm     = xb * (~src_mask)[:, :, None]     # zero padded tokens
    left   [b,g,d] = sum_{s<=g} xm[b,s,d]     # masked prefix sums
    right  [b,g,d] = sum_{s>g}  xm[b,s,d]
    out    = concat([left, right], axis=2)    # (B, G, 2D)

Strategy: pure data parallel over B across 8 cores (64 batches each).
The pad mask is folded into x on the host (exact: multiply by 0/1), and
x is cast to bf16 on the host — halves input HBM traffic; the 0/1
triangular weights are exact in bf16 so only x's mantissa truncation
matters (~2e-3 rel err, gate is 2e-2). Per batch the prefix/suffix sums
are computed on the TensorEngine as two matmuls against constant 0/1
triangular matrices (contraction over S=128 on partitions, f32 PSUM
accumulate), then PSUM->SBUF copies (DVE for even batches, ACT for odd)
cast the result to bf16. The host upcasts the returned bf16 block.

DMA (the roofline resource: ~25 MB/core at ~358 GB/s HBM): reads use
HWDGE (scalar/ACT ring), which sprays each transfer's per-partition
descriptors across all 16 SDMA engines by destination SBUF port
(~300 GB/s measured). HWDGE does NOT spray HBM-destined writes - all
descriptors land on one engine (~27 GB/s/ring, measured) - so output
writes go through SWDGE (gpsimd): each DMA binds to one engine
(~20 GB/s effective incl. the per-512B 4-byte completion writes), and
Tile's 8 DMASW sem lanes keep 8 in flight -> ~160 GB/s aggregate.
Output is written g-major (out[g, b, 2D]) so one partition row is a
128 KB contiguous DRAM run: OUT_CHUNK=4 batches -> 127 descriptors of
8 KB per DMA, 16 DMAs pipelined over the 8 lanes.
"""

import numpy as np
import ml_dtypes

import concourse.bass as bass
import concourse.mybir as mybir
from concourse import bacc
from concourse.tile import TileContext
from concourse.bass_utils import run_bass_kernel_spmd

S, B, D = 128, 512, 512
G = S - 1
N_CORES = 8
BL = B // N_CORES  # 64 batches per core

IN_CHUNK = 16  # batches per input DMA  (16 KB per-partition descriptors)
OUT_CHUNK = 4  # batches per output DMA (8 KB per-partition descriptors)

_NC_CACHE = None


def _build_bass() -> bass.Bass:
    nc = bacc.Bacc()
    f32 = mybir.dt.float32
    bf16 = mybir.dt.bfloat16

    x_in = nc.declare_dram_parameter("x", [S, BL, D], bf16, isOutput=False)
    # tri[:, 0:128] = upper (incl diag)  tri[s,g] = 1 if s <= g  -> prefix sums
    # tri[:, 128:256] = strictly lower   tri[s,g] = 1 if s >  g  -> suffix sums
    t_in = nc.declare_dram_parameter("tri", [S, 2 * S], bf16, isOutput=False)
    # g-major per-core output: partition row g maps to a contiguous DRAM run,
    # host transposes (G, BL, 2D) -> (BL, G, 2D) while gathering.
    out = nc.declare_dram_parameter("out", [G, BL, 2 * D], bf16, isOutput=True)

    with TileContext(nc) as tc:
        with (
            tc.tile_pool(name="const", bufs=1) as cpool,
            tc.tile_pool(name="xin", bufs=2) as xpool,
            tc.tile_pool(name="outs", bufs=8) as opool,
            tc.tile_pool(name="psum", bufs=2, space="PSUM") as ppool,
        ):
            tri = cpool.tile([S, 2 * S], bf16)
            nc.sync.dma_start(out=tri[:], in_=t_in[:])
            ut = tri[:, 0:S]        # (128, 128) stationary, left sums
            lt = tri[:, S : 2 * S]  # (128, 128) stationary, right sums

            def per_pair(xt, ot, j, k):
                """2 batches (j, j+1) of xt -> slots (k, k+1) of ot.

                One 4-bank PSUM tile takes all 4 matmuls; DVE copies batch j,
                ACT copies batch j+1 - one copy op per batch total.
                """
                ps = ppool.tile([S, 4, D], f32)  # 4 adjacent banks
                for h, (b, tri_) in enumerate(
                    [(j, ut), (j, lt), (j + 1, ut), (j + 1, lt)]
                ):
                    nc.tensor.matmul(out=ps[:, h, :], lhsT=tri_, rhs=xt[:, b, :],
                                     start=True, stop=True)
                nc.vector.tensor_copy(
                    out=ot[0:G, k, :].rearrange("g (h d) -> g h d", h=2),
                    in_=ps[0:G, 0:2, :],
                )
                nc.scalar.activation(
                    out=ot[0:G, k + 1, :].rearrange("g (h d) -> g h d", h=2),
                    in_=ps[0:G, 2:4, :],
                    func=mybir.ActivationFunctionType.Copy,
                )

            for b0 in range(0, BL, IN_CHUNK):
                xt = xpool.tile([S, IN_CHUNK, D], bf16)
                # input loads on the ACT HWDGE ring (sprays all 16 engines)
                nc.scalar.dma_start(
                    out=xt[:], in_=x_in[:, b0 : b0 + IN_CHUNK, :])
                for o0 in range(b0, b0 + IN_CHUNK, OUT_CHUNK):
                    ot = opool.tile([S, OUT_CHUNK, 2 * D], bf16)
                    for j in range(0, OUT_CHUNK, 2):
                        per_pair(xt, ot, o0 - b0 + j, j)
                    nc.gpsimd.dma_start(
                        out=out[:, o0 : o0 + OUT_CHUNK, :], in_=ot[0:G, :, :],
                    )
    nc.finalize()  # runs the Bacc pass pipeline (reg alloc, wait splitting)
    return nc


def _get_nc() -> bass.Bass:
    global _NC_CACHE
    if _NC_CACHE is None:
        _NC_CACHE = _build_bass()
    return _NC_CACHE


def _make_in_maps(x: np.ndarray, src_mask: np.ndarray) -> list[dict]:
    x = np.asarray(x, dtype=np.float32)
    src_mask = np.asarray(src_mask)
    assert x.shape == (S, B, D), x.shape
    assert src_mask.shape == (B, S), src_mask.shape

    valid = (~src_mask.astype(bool)).astype(np.float32).T  # (S, B)
    xm = (x * valid[:, :, None]).astype(ml_dtypes.bfloat16)
    tri = np.concatenate(
        [
            np.triu(np.ones((S, S), np.float32)),       # s <= g
            np.tril(np.ones((S, S), np.float32), -1),   # s >  g
        ],
        axis=1,
    ).astype(ml_dtypes.bfloat16)

    in_maps = []
    for i in range(N_CORES):
        sl = slice(i * BL, (i + 1) * BL)
        in_maps.append(
            {
                "x": np.ascontiguousarray(xm[:, sl, :]),
                "tri": tri,
            }
        )
    return in_maps


def _assemble(results: list[dict]) -> np.ndarray:
    full = np.empty((B, G, 2 * D), dtype=np.float32)
    for i in range(N_CORES):
        full[i * BL : (i + 1) * BL] = (
            results[i]["out"].transpose(1, 0, 2).astype(np.float32)
        )
    return full


def kernel(x: np.ndarray, src_mask: np.ndarray) -> np.ndarray:
    in_maps = _make_in_maps(x, src_mask)
    res = run_bass_kernel_spmd(_get_nc(), in_maps, core_ids=list(range(N_CORES)))
    return _assemble(res.results)
